# revision 7
# baseline (speedup 1.0000x reference)
"""Trainium2 Bass kernel for multi-head causal attention.

Problem (hardcoded): x [2, 2048, 1024] fp32, w_qkv [1024, 3072], w_out [1024, 1024].
  qkv = x @ w_qkv; per-head causal softmax attention (16 heads, d=64);
  out = attn_out @ w_out.

Sharding: 8 cores = (2 batches) x (4 head-groups of 4 heads).
Each core computes, for its batch b and heads 4g..4g+3:
  - Q^T, K^T [256, 2048] and V [2048, 256] from x[b]^T (host-pretransposed)
  - flash-style causal attention entirely on-chip (S^T layout, exp on ACT,
    causal mask via gpsimd affine_select, rowsum via ones-column matmul)
  - partial out-projection y_part = attn_out_g @ w_out[256g:256g+256, :]
Host gathers: y[b] = sum_g y_part[4b+g]. Matmuls run as float32r (full PE
rate, ~1e-4 relative error).
"""
import numpy as np

import concourse.bass as bass
from concourse import bacc
import concourse.mybir as mybir
import concourse.tile as tile

F32 = mybir.dt.float32
F32R = mybir.dt.float32r
AF = mybir.ActivationFunctionType

B, T, C = 2, 2048, 1024
H_TOT, D = 16, 64
HL = 4             # heads per core
DL = HL * D        # 256 local channels
NJ = 4             # q-chunks of 512
NKT = 16           # k-tiles of 128
NCT = 8            # c-tiles of 128 (contraction over C)
SM_SCALE = 1.0 / np.sqrt(D)

_CACHE = {}


def build_nc():
    nc = bacc.Bacc("TRN2", target_bir_lowering=False)
    xt = nc.dram_tensor("xt", [C, T], F32R, kind="ExternalInput")
    wqk = nc.dram_tensor("wqk", [C, 2 * DL], F32R, kind="ExternalInput")
    wv = nc.dram_tensor("wv", [C, DL], F32R, kind="ExternalInput")
    wo = nc.dram_tensor("wo", [DL, C], F32R, kind="ExternalInput")
    ones_c = nc.dram_tensor("ones_c", [128, HL], F32R, kind="ExternalInput")
    y = nc.dram_tensor("y", [T, C], F32, kind="ExternalOutput")

    with tile.TileContext(nc) as tc:
        with tc.tile_pool(name="persist", bufs=1) as persist, \
             tc.tile_pool(name="dram", bufs=1, space="DRAM") as drampool:
            # persistent tiles
            qk_tiles = [persist.tile([128, T], F32R, tag=f"qk{m}", name=f"qk{m}") for m in range(4)]
            qt_t, kt_t = qk_tiles[0:2], qk_tiles[2:4]
            v_sb = [persist.tile([128, HL, D + 1], F32R, tag=f"v{t}", name=f"v{t}")
                    for t in range(NKT)]
            rc_dram = drampool.tile([16, 512], F32)

            # ---------------- Phase 1: QKV projections ----------------
            with tc.tile_pool(name="load", bufs=1) as load, \
                 tc.tile_pool(name="p1ps", bufs=4, space="PSUM") as p1ps:
                xt_sb = []
                wqk_sb = []
                wv_sb = []
                for c in range(NCT):
                    xts = load.tile([128, T], F32R, tag=f"xt{c}", name=f"xt{c}")
                    nc.sync.dma_start(out=xts[:], in_=xt[128 * c:128 * (c + 1), :])
                    xt_sb.append(xts)
                    wqks = load.tile([128, 2 * DL], F32R, tag=f"wqk{c}", name=f"wqk{c}")
                    nc.sync.dma_start(out=wqks[:], in_=wqk[128 * c:128 * (c + 1), :])
                    wqk_sb.append(wqks)
                    wvs = load.tile([128, DL], F32R, tag=f"wv{c}", name=f"wv{c}")
                    nc.sync.dma_start(out=wvs[:], in_=wv[128 * c:128 * (c + 1), :])
                    wv_sb.append(wvs)

                # Q^T / K^T: [128 (2 heads x 64d), 2048 t]
                for m in range(4):
                    dest = qk_tiles[m]
                    for j in range(NJ):
                        ps = p1ps.tile([128, 512], F32, tag="acc")
                        for c in range(NCT):
                            nc.tensor.matmul(
                                ps[:],
                                wqk_sb[c][:, 128 * m:128 * (m + 1)],
                                xt_sb[c][:, 512 * j:512 * (j + 1)],
                                start=(c == 0), stop=(c == NCT - 1))
                        nc.vector.tensor_copy(
                            dest[:, 512 * j:512 * (j + 1)], ps[:])

                # V natural: [128 t, 4 heads x 64 d] + ones column
                for t in range(NKT):
                    vps = p1ps.tile([128, DL], F32, tag="vacc")
                    for c in range(NCT):
                        nc.tensor.matmul(
                            vps[:],
                            xt_sb[c][:, 128 * t:128 * (t + 1)],
                            wv_sb[c][:],
                            start=(c == 0), stop=(c == NCT - 1))
                    nc.vector.tensor_copy(
                        v_sb[t][:, :, 0:D],
                        vps[:].rearrange("p (h d) -> p h d", h=HL))
                    nc.sync.dma_start(
                        out=v_sb[t][:, :, D:D + 1],
                        in_=ones_c[:, :].rearrange("p (h o) -> p h o", o=1))

            # ---------------- Phases 2-4 ----------------
            with tc.tile_pool(name="late", bufs=1) as late, \
                 tc.tile_pool(name="pexp", bufs=10) as pexp:
                at_t = [late.tile([128, T], F32R, tag=f"at{p}", name=f"at{p}") for p in range(2)]
                rs_w = late.tile([16, 512], F32, tag="rs")
                wo_sb = [late.tile([128, C], F32R, tag=f"wo{i}", name=f"wo{i}") for i in range(2)]
                for i in range(2):
                    nc.sync.dma_start(out=wo_sb[i][:],
                                      in_=wo[128 * i:128 * (i + 1), :])

                # Phase 2: attention
                with tc.tile_pool(name="p2s", bufs=6, space="PSUM") as p2s, \
                     tc.tile_pool(name="rssc", bufs=4) as rsscp, \
                     tc.tile_pool(name="p2ot", bufs=1, space="PSUM") as p2ot:
                    for pair in range(2):
                        qt, kt = qt_t[pair], kt_t[pair]
                        for j in range(NJ):
                            nkt = 4 * (j + 1)
                            ot_ps = [p2ot.tile([65, 512], F32, tag=f"ot{h2}", name=f"ot{h2}")
                                     for h2 in range(2)]
                            for ktg in range(j + 1):          # groups of 4 k-tiles
                                for h2 in range(2):
                                    base = 64 * h2
                                    h = 2 * pair + h2
                                    s_group = []
                                    for kk in range(4):
                                        ktt = 4 * ktg + kk
                                        s_ps = p2s.tile([128, 512], F32, tag="s")
                                        nc.tensor.matmul(
                                            s_ps[:],
                                            kt[base:base + 64, 128 * ktt:128 * (ktt + 1)],
                                            qt[base:base + 64, 512 * j:512 * (j + 1)],
                                            start=True, stop=True)
                                        s_group.append(s_ps)
                                    p_group = []
                                    for kk in range(4):
                                        p_t = pexp.tile([128, 512], F32R, tag="p")
                                        nc.scalar.activation(
                                            p_t[:], s_group[kk][:], AF.Exp,
                                            scale=float(SM_SCALE))
                                        p_group.append(p_t)
                                    if ktg == j:              # diagonal group
                                        for kk in range(4):
                                            nc.gpsimd.affine_select(
                                                out=p_group[kk][:],
                                                in_=p_group[kk][:],
                                                compare_op=mybir.AluOpType.is_ge,
                                                fill=0.0, base=-128 * kk,
                                                pattern=[[1, 512]],
                                                channel_multiplier=-1)
                                    for kk in range(4):
                                        ktt = 4 * ktg + kk
                                        nc.tensor.matmul(
                                            ot_ps[h2][:],
                                            v_sb[ktt][:, h, 0:D + 1],
                                            p_group[kk][:],
                                            start=(ktt == 0), stop=(ktt == nkt - 1))
                            for h2 in range(2):
                                h = 2 * pair + h2
                                nc.vector.tensor_copy(
                                    at_t[pair][64 * h2:64 * h2 + 64,
                                               512 * j:512 * (j + 1)],
                                    ot_ps[h2][0:64, :])
                                idx = 4 * h + j
                                # engine APs need 32-aligned partition base:
                                # stage rowsum at partition 64, DMA-repack.
                                rssc = rsscp.tile([65, 512], F32, tag="rssc")
                                nc.vector.tensor_copy(
                                    rssc[64:65, :], ot_ps[h2][64:65, :])
                                nc.sync.dma_start(
                                    out=rs_w[idx:idx + 1, :],
                                    in_=rssc[64:65, :])

                # Phase 3: normalize
                rc_w = late.tile([16, 512], F32, tag="rc")
                rc_scr = late.tile([16, 512], F32, tag="rcs")
                nc.vector.reciprocal_approx_accurate(rc_w[:], rs_w[:], rc_scr[:])
                nc.sync.dma_start(out=rc_dram[:, :], in_=rc_w[:])
                bc_t = [late.tile([128, T], F32, tag=f"bc{p}", name=f"bc{p}") for p in range(2)]
                for pair in range(2):
                    for h2 in range(2):
                        h = 2 * pair + h2
                        seg = rc_dram[4 * h:4 * h + 4, :]
                        bcast = bass.AP(
                            tensor=seg.tensor, offset=seg.offset,
                            ap=[[0, 64]] + list(seg.ap))
                        nc.sync.dma_start(
                            out=bc_t[pair][64 * h2:64 * h2 + 64, :].rearrange(
                                "p (a b) -> p a b", a=4),
                            in_=bcast)
                    nc.vector.tensor_mul(
                        at_t[pair][:], at_t[pair][:], bc_t[pair][:])

                # Phase 4: out-projection (partial)
                with tc.tile_pool(name="p4y", bufs=4, space="PSUM") as p4y, \
                     tc.tile_pool(name="ysb", bufs=4) as ysbp:
                    for t in range(NKT):
                        for oc in range(2):
                            yps = p4y.tile([128, 512], F32, tag="y")
                            for i in range(2):
                                nc.tensor.matmul(
                                    yps[:],
                                    at_t[i][:, 128 * t:128 * (t + 1)],
                                    wo_sb[i][:, 512 * oc:512 * (oc + 1)],
                                    start=(i == 0), stop=(i == 1))
                            ysb = ysbp.tile([128, 512], F32, tag="ysb")
                            if oc == 0:
                                nc.vector.tensor_copy(ysb[:], yps[:])
                            else:
                                nc.scalar.copy(out=ysb[:], in_=yps[:])
                            nc.sync.dma_start(
                                out=y[128 * t:128 * (t + 1),
                                      512 * oc:512 * (oc + 1)],
                                in_=ysb[:])
    nc.compile()
    return nc


def _get_runner():
    """Compile once; return a callable(in_maps) -> list of per-core out dicts."""
    if "runner" in _CACHE:
        return _CACHE["runner"]
    import jax
    import jax.numpy as jnp
    from jax.sharding import Mesh, PartitionSpec
    from jax.experimental.shard_map import shard_map
    from concourse import bass2jax

    nc = build_nc()
    bass2jax.install_neuronx_cc_hook()

    partition_name = (nc.partition_id_tensor.name
                      if nc.partition_id_tensor else None)
    in_names, out_names, out_avals, zero_outs = [], [], [], []
    for alloc in nc.m.functions[0].allocations:
        if not isinstance(alloc, mybir.MemoryLocationSet):
            continue
        name = alloc.memorylocations[0].name
        if alloc.kind == "ExternalInput":
            if name != partition_name:
                in_names.append(name)
        elif alloc.kind == "ExternalOutput":
            out_names.append(name)
            shape = tuple(alloc.tensor_shape)
            dtype = mybir.dt.np(alloc.dtype)
            out_avals.append(jax.core.ShapedArray(shape, dtype))
            zero_outs.append(np.zeros(shape, dtype))
    n_params = len(in_names)
    n_outs = len(out_avals)
    all_in_names = list(in_names) + list(out_names)
    if partition_name is not None:
        all_in_names.append(partition_name)
    donate = tuple(range(n_params, n_params + n_outs))

    def _body(*args):
        operands = list(args)
        if partition_name is not None:
            operands.append(bass2jax.partition_id_tensor())
        outs = bass2jax._bass_exec_p.bind(
            *operands,
            out_avals=tuple(out_avals),
            in_names=tuple(all_in_names),
            out_names=tuple(out_names),
            lowering_input_output_aliases=(),
            sim_require_finite=True,
            sim_require_nnan=True,
            nc=nc,
        )
        return tuple(outs)

    n_cores = 8
    devices = jax.devices()[:n_cores]
    mesh = Mesh(np.asarray(devices), ("core",))
    in_specs = (PartitionSpec("core"),) * (n_params + n_outs)
    out_specs = (PartitionSpec("core"),) * n_outs
    sharded = jax.jit(
        shard_map(_body, mesh=mesh, in_specs=in_specs, out_specs=out_specs,
                  check_rep=False),
        donate_argnums=donate, keep_unused=True)

    def run(in_maps):
        per_core = [[np.asarray(m[name]) for name in in_names] for m in in_maps]
        concat_in = [np.concatenate([per_core[c][i] for c in range(n_cores)],
                                    axis=0) for i in range(n_params)]
        concat_zeros = [np.zeros((n_cores * z.shape[0], *z.shape[1:]), z.dtype)
                        for z in zero_outs]
        out_arrs = sharded(*concat_in, *concat_zeros)
        return [
            {name: np.asarray(out_arrs[i]).reshape(n_cores,
                                                   *out_avals[i].shape)[c]
             for i, name in enumerate(out_names)}
            for c in range(n_cores)
        ]

    _CACHE["runner"] = run
    return run


def _prep_in_maps(x, w_qkv, w_out):
    x = np.asarray(x, dtype=np.float32)
    w_qkv = np.asarray(w_qkv, dtype=np.float32)
    w_out = np.asarray(w_out, dtype=np.float32)
    in_maps = []
    xts = [np.ascontiguousarray(x[b].T) for b in range(B)]
    for core in range(8):
        b, g = divmod(core, 4)
        cl, ch = 256 * g, 256 * g + 256
        wqk = np.ascontiguousarray(
            np.concatenate([w_qkv[:, cl:ch], w_qkv[:, C + cl:C + ch]], axis=1))
        wv = np.ascontiguousarray(w_qkv[:, 2 * C + cl:2 * C + ch])
        wo = np.ascontiguousarray(w_out[cl:ch, :])
        in_maps.append({"xt": xts[b], "wqk": wqk, "wv": wv, "wo": wo,
                        "ones_c": np.ones((128, HL), dtype=np.float32)})
    return in_maps


def kernel(x, w_qkv, w_out):
    run = _get_runner()
    in_maps = _prep_in_maps(x, w_qkv, w_out)
    results = run(in_maps)
    y = np.zeros((B, T, C), dtype=np.float32)
    for core in range(8):
        b = core // 4
        y[b] += results[core]["y"]
    return y


if __name__ == "__main__":
    rng = np.random.default_rng(0)
    x = rng.standard_normal((B, T, C)).astype(np.float32)
    w_qkv = (rng.standard_normal((C, 3 * C)) / np.sqrt(C)).astype(np.float32)
    w_out = (rng.standard_normal((C, C)) / np.sqrt(C)).astype(np.float32)
    y = kernel(x=x, w_qkv=w_qkv, w_out=w_out)
    print("kernel ran, y:", y.shape, y.dtype, float(np.abs(y).max()))


# revision 9
# speedup vs baseline: 303.7190x; 303.7190x over previous
"""Trainium2 Bass kernel for multi-head causal attention.

Problem (hardcoded): x [2, 2048, 1024] fp32, w_qkv [1024, 3072], w_out [1024, 1024].
  qkv = x @ w_qkv; per-head causal softmax attention (16 heads, d=64);
  out = attn_out @ w_out.

Sharding: 8 cores = (2 batches) x (4 head-groups of 4 heads).
Each core computes, for its batch b and heads 4g..4g+3:
  - Q^T, K^T [256, 2048] and V [2048, 256] from x[b]^T (host-pretransposed)
  - flash-style causal attention entirely on-chip (S^T layout, exp on ACT,
    causal mask via gpsimd affine_select, rowsum via ones-column matmul)
  - partial out-projection y_part = attn_out_g @ w_out[256g:256g+256, :]
Host gathers: y[b] = sum_g y_part[4b+g]. Matmuls run as float32r (full PE
rate, ~1e-4 relative error).
"""
import numpy as np

import concourse.bass as bass
from concourse import bacc
import concourse.mybir as mybir
import concourse.tile as tile

F32 = mybir.dt.float32
F32R = mybir.dt.float32r
AF = mybir.ActivationFunctionType

B, T, C = 2, 2048, 1024
H_TOT, D = 16, 64
HL = 4             # heads per core
DL = HL * D        # 256 local channels
NJ = 4             # q-chunks of 512
NKT = 16           # k-tiles of 128
NCT = 8            # c-tiles of 128 (contraction over C)
SM_SCALE = 1.0 / np.sqrt(D)

_CACHE = {}


def build_nc(reps=1):
    nc = bacc.Bacc("TRN2", target_bir_lowering=False)
    xt = nc.dram_tensor("xt", [C, T], F32R, kind="ExternalInput")
    wqk = nc.dram_tensor("wqk", [C, 2 * DL], F32R, kind="ExternalInput")
    wv = nc.dram_tensor("wv", [C, DL], F32R, kind="ExternalInput")
    wo = nc.dram_tensor("wo", [DL, C], F32R, kind="ExternalInput")
    ones_c = nc.dram_tensor("ones_c", [128, HL], F32R, kind="ExternalInput")
    y = nc.dram_tensor("y", [T, C], F32, kind="ExternalOutput")

    with tile.TileContext(nc) as tc:
      for _rep in range(reps):
        with tc.tile_pool(name="persist", bufs=1) as persist, \
             tc.tile_pool(name="dram", bufs=1, space="DRAM") as drampool:
            # persistent tiles
            qk_tiles = [persist.tile([128, T], F32R, tag=f"qk{m}", name=f"qk{m}") for m in range(4)]
            qt_t, kt_t = qk_tiles[0:2], qk_tiles[2:4]
            v_sb = [persist.tile([128, HL, D + 1], F32R, tag=f"v{t}", name=f"v{t}")
                    for t in range(NKT)]
            rc_dram = drampool.tile([16, 512], F32)

            # ---------------- Phase 1: QKV projections ----------------
            with tc.tile_pool(name="load", bufs=1) as load, \
                 tc.tile_pool(name="p1ps", bufs=4, space="PSUM") as p1ps:
                xt_sb = []
                wqk_sb = []
                wv_sb = []
                for c in range(NCT):
                    xts = load.tile([128, T], F32R, tag=f"xt{c}", name=f"xt{c}")
                    nc.sync.dma_start(out=xts[:], in_=xt[128 * c:128 * (c + 1), :])
                    xt_sb.append(xts)
                    wqks = load.tile([128, 2 * DL], F32R, tag=f"wqk{c}", name=f"wqk{c}")
                    nc.sync.dma_start(out=wqks[:], in_=wqk[128 * c:128 * (c + 1), :])
                    wqk_sb.append(wqks)
                    wvs = load.tile([128, DL], F32R, tag=f"wv{c}", name=f"wv{c}")
                    nc.sync.dma_start(out=wvs[:], in_=wv[128 * c:128 * (c + 1), :])
                    wv_sb.append(wvs)

                # Q^T / K^T: [128 (2 heads x 64d), 2048 t]
                for m in range(4):
                    dest = qk_tiles[m]
                    for j in range(NJ):
                        ps = p1ps.tile([128, 512], F32, tag="acc")
                        for c in range(NCT):
                            nc.tensor.matmul(
                                ps[:],
                                wqk_sb[c][:, 128 * m:128 * (m + 1)],
                                xt_sb[c][:, 512 * j:512 * (j + 1)],
                                start=(c == 0), stop=(c == NCT - 1))
                        nc.vector.tensor_copy(
                            dest[:, 512 * j:512 * (j + 1)], ps[:])

                # V natural: [128 t, 4 heads x 64 d] + ones column
                for t in range(NKT):
                    vps = p1ps.tile([128, DL], F32, tag="vacc")
                    for c in range(NCT):
                        nc.tensor.matmul(
                            vps[:],
                            xt_sb[c][:, 128 * t:128 * (t + 1)],
                            wv_sb[c][:],
                            start=(c == 0), stop=(c == NCT - 1))
                    nc.vector.tensor_copy(
                        v_sb[t][:, :, 0:D],
                        vps[:].rearrange("p (h d) -> p h d", h=HL))
                    nc.sync.dma_start(
                        out=v_sb[t][:, :, D:D + 1],
                        in_=ones_c[:, :].rearrange("p (h o) -> p h o", o=1))

            # ---------------- Phases 2-4 ----------------
            with tc.tile_pool(name="late", bufs=1) as late, \
                 tc.tile_pool(name="pexp", bufs=10) as pexp:
                at_t = [late.tile([128, T], F32R, tag=f"at{p}", name=f"at{p}") for p in range(2)]
                rs_w = late.tile([16, 512], F32, tag="rs")
                wo_sb = [late.tile([128, C], F32R, tag=f"wo{i}", name=f"wo{i}") for i in range(2)]
                for i in range(2):
                    nc.sync.dma_start(out=wo_sb[i][:],
                                      in_=wo[128 * i:128 * (i + 1), :])

                # Phase 2: attention
                with tc.tile_pool(name="p2s", bufs=6, space="PSUM") as p2s, \
                     tc.tile_pool(name="rssc", bufs=4) as rsscp, \
                     tc.tile_pool(name="p2ot", bufs=1, space="PSUM") as p2ot:
                    for pair in range(2):
                        qt, kt = qt_t[pair], kt_t[pair]
                        for j in range(NJ):
                            nkt = 4 * (j + 1)
                            ot_ps = [p2ot.tile([65, 512], F32, tag=f"ot{h2}", name=f"ot{h2}")
                                     for h2 in range(2)]
                            for ktg in range(j + 1):          # groups of 4 k-tiles
                                for h2 in range(2):
                                    base = 64 * h2
                                    h = 2 * pair + h2
                                    s_group = []
                                    for kk in range(4):
                                        ktt = 4 * ktg + kk
                                        s_ps = p2s.tile([128, 512], F32, tag="s")
                                        nc.tensor.matmul(
                                            s_ps[:],
                                            kt[base:base + 64, 128 * ktt:128 * (ktt + 1)],
                                            qt[base:base + 64, 512 * j:512 * (j + 1)],
                                            start=True, stop=True)
                                        s_group.append(s_ps)
                                    p_group = []
                                    for kk in range(4):
                                        p_t = pexp.tile([128, 512], F32R, tag="p")
                                        nc.scalar.activation(
                                            p_t[:], s_group[kk][:], AF.Exp,
                                            scale=float(SM_SCALE))
                                        p_group.append(p_t)
                                    if ktg == j:              # diagonal group
                                        for kk in range(4):
                                            nc.gpsimd.affine_select(
                                                out=p_group[kk][:],
                                                in_=p_group[kk][:],
                                                compare_op=mybir.AluOpType.is_ge,
                                                fill=0.0, base=-128 * kk,
                                                pattern=[[1, 512]],
                                                channel_multiplier=-1)
                                    for kk in range(4):
                                        ktt = 4 * ktg + kk
                                        nc.tensor.matmul(
                                            ot_ps[h2][:],
                                            v_sb[ktt][:, h, 0:D + 1],
                                            p_group[kk][:],
                                            start=(ktt == 0), stop=(ktt == nkt - 1))
                            for h2 in range(2):
                                h = 2 * pair + h2
                                nc.vector.tensor_copy(
                                    at_t[pair][64 * h2:64 * h2 + 64,
                                               512 * j:512 * (j + 1)],
                                    ot_ps[h2][0:64, :])
                                idx = 4 * h + j
                                # engine APs need 32-aligned partition base:
                                # stage rowsum at partition 64, DMA-repack.
                                rssc = rsscp.tile([65, 512], F32, tag="rssc")
                                nc.vector.tensor_copy(
                                    rssc[64:65, :], ot_ps[h2][64:65, :])
                                nc.sync.dma_start(
                                    out=rs_w[idx:idx + 1, :],
                                    in_=rssc[64:65, :])

                # Phase 3: normalize
                rc_w = late.tile([16, 512], F32, tag="rc")
                rc_scr = late.tile([16, 512], F32, tag="rcs")
                nc.vector.reciprocal_approx_accurate(rc_w[:], rs_w[:], rc_scr[:])
                nc.sync.dma_start(out=rc_dram[:, :], in_=rc_w[:])
                bc_t = [late.tile([128, T], F32, tag=f"bc{p}", name=f"bc{p}") for p in range(2)]
                for pair in range(2):
                    for h2 in range(2):
                        h = 2 * pair + h2
                        seg = rc_dram[4 * h:4 * h + 4, :]
                        bcast = bass.AP(
                            tensor=seg.tensor, offset=seg.offset,
                            ap=[[0, 64]] + list(seg.ap))
                        nc.sync.dma_start(
                            out=bc_t[pair][64 * h2:64 * h2 + 64, :].rearrange(
                                "p (a b) -> p a b", a=4),
                            in_=bcast)
                    nc.vector.tensor_mul(
                        at_t[pair][:], at_t[pair][:], bc_t[pair][:])

                # Phase 4: out-projection (partial)
                with tc.tile_pool(name="p4y", bufs=4, space="PSUM") as p4y, \
                     tc.tile_pool(name="ysb", bufs=4) as ysbp:
                    for t in range(NKT):
                        for oc in range(2):
                            yps = p4y.tile([128, 512], F32, tag="y")
                            for i in range(2):
                                nc.tensor.matmul(
                                    yps[:],
                                    at_t[i][:, 128 * t:128 * (t + 1)],
                                    wo_sb[i][:, 512 * oc:512 * (oc + 1)],
                                    start=(i == 0), stop=(i == 1))
                            ysb = ysbp.tile([128, 512], F32, tag="ysb")
                            if oc == 0:
                                nc.vector.tensor_copy(ysb[:], yps[:])
                            else:
                                nc.scalar.copy(out=ysb[:], in_=yps[:])
                            nc.sync.dma_start(
                                out=y[128 * t:128 * (t + 1),
                                      512 * oc:512 * (oc + 1)],
                                in_=ysb[:])
    nc.compile()
    return nc


def _get_runner(reps=1):
    """Compile once; return a callable(in_maps) -> list of per-core out dicts."""
    key = ("runner", reps)
    if key in _CACHE:
        return _CACHE[key]
    import jax
    import jax.numpy as jnp
    from jax.sharding import Mesh, PartitionSpec
    from jax.experimental.shard_map import shard_map
    from concourse import bass2jax

    nc = build_nc(reps)
    bass2jax.install_neuronx_cc_hook()

    partition_name = (nc.partition_id_tensor.name
                      if nc.partition_id_tensor else None)
    in_names, out_names, out_avals, zero_outs = [], [], [], []
    for alloc in nc.m.functions[0].allocations:
        if not isinstance(alloc, mybir.MemoryLocationSet):
            continue
        name = alloc.memorylocations[0].name
        if alloc.kind == "ExternalInput":
            if name != partition_name:
                in_names.append(name)
        elif alloc.kind == "ExternalOutput":
            out_names.append(name)
            shape = tuple(alloc.tensor_shape)
            dtype = mybir.dt.np(alloc.dtype)
            out_avals.append(jax.core.ShapedArray(shape, dtype))
            zero_outs.append(np.zeros(shape, dtype))
    n_params = len(in_names)
    n_outs = len(out_avals)
    all_in_names = list(in_names) + list(out_names)
    if partition_name is not None:
        all_in_names.append(partition_name)
    donate = tuple(range(n_params, n_params + n_outs))

    def _body(*args):
        operands = list(args)
        if partition_name is not None:
            operands.append(bass2jax.partition_id_tensor())
        outs = bass2jax._bass_exec_p.bind(
            *operands,
            out_avals=tuple(out_avals),
            in_names=tuple(all_in_names),
            out_names=tuple(out_names),
            lowering_input_output_aliases=(),
            sim_require_finite=True,
            sim_require_nnan=True,
            nc=nc,
        )
        return tuple(outs)

    n_cores = 8
    devices = jax.devices()[:n_cores]
    mesh = Mesh(np.asarray(devices), ("core",))
    in_specs = (PartitionSpec("core"),) * (n_params + n_outs)
    out_specs = (PartitionSpec("core"),) * n_outs
    sharded = jax.jit(
        shard_map(_body, mesh=mesh, in_specs=in_specs, out_specs=out_specs,
                  check_rep=False),
        donate_argnums=donate, keep_unused=True)

    def run(in_maps):
        per_core = [[np.asarray(m[name]) for name in in_names] for m in in_maps]
        concat_in = [np.concatenate([per_core[c][i] for c in range(n_cores)],
                                    axis=0) for i in range(n_params)]
        concat_zeros = [np.zeros((n_cores * z.shape[0], *z.shape[1:]), z.dtype)
                        for z in zero_outs]
        out_arrs = sharded(*concat_in, *concat_zeros)
        return [
            {name: np.asarray(out_arrs[i]).reshape(n_cores,
                                                   *out_avals[i].shape)[c]
             for i, name in enumerate(out_names)}
            for c in range(n_cores)
        ]

    _CACHE[key] = run
    return run


def _get_bench(reps=1):
    """Zero-transfer bench callable: inputs pre-placed on device, outputs
    left on device (block_until_ready only). No donation."""
    key = ("bench", reps)
    if key in _CACHE:
        return _CACHE[key]
    import jax
    from jax.sharding import Mesh, PartitionSpec, NamedSharding
    from jax.experimental.shard_map import shard_map
    from concourse import bass2jax

    nc = build_nc(reps)
    bass2jax.install_neuronx_cc_hook()
    partition_name = (nc.partition_id_tensor.name
                      if nc.partition_id_tensor else None)
    in_names, out_names, out_avals, zero_outs = [], [], [], []
    for alloc in nc.m.functions[0].allocations:
        if not isinstance(alloc, mybir.MemoryLocationSet):
            continue
        name = alloc.memorylocations[0].name
        if alloc.kind == "ExternalInput":
            if name != partition_name:
                in_names.append(name)
        elif alloc.kind == "ExternalOutput":
            out_names.append(name)
            shape = tuple(alloc.tensor_shape)
            dtype = mybir.dt.np(alloc.dtype)
            out_avals.append(jax.core.ShapedArray(shape, dtype))
            zero_outs.append(np.zeros(shape, dtype))
    n_params = len(in_names)
    all_in_names = list(in_names) + list(out_names)
    if partition_name is not None:
        all_in_names.append(partition_name)

    def _body(*args):
        operands = list(args)
        if partition_name is not None:
            operands.append(bass2jax.partition_id_tensor())
        outs = bass2jax._bass_exec_p.bind(
            *operands,
            out_avals=tuple(out_avals),
            in_names=tuple(all_in_names),
            out_names=tuple(out_names),
            lowering_input_output_aliases=(),
            sim_require_finite=True,
            sim_require_nnan=True,
            nc=nc,
        )
        return tuple(outs)

    n_cores = 8
    devices = jax.devices()[:n_cores]
    mesh = Mesh(np.asarray(devices), ("core",))
    nouts = len(out_names)
    in_specs = (PartitionSpec("core"),) * (n_params + nouts)
    out_specs = (PartitionSpec("core"),) * nouts
    sharded = jax.jit(
        shard_map(_body, mesh=mesh, in_specs=in_specs, out_specs=out_specs,
                  check_rep=False),
        keep_unused=True)
    shard = NamedSharding(mesh, PartitionSpec("core"))

    def make_args(in_maps):
        per_core = [[np.asarray(m[name]) for name in in_names]
                    for m in in_maps]
        concat_in = [np.concatenate([per_core[c][i] for c in range(n_cores)],
                                    axis=0) for i in range(n_params)]
        concat_zeros = [np.zeros((n_cores * z.shape[0], *z.shape[1:]),
                                 z.dtype) for z in zero_outs]
        return [jax.device_put(a, shard) for a in concat_in + concat_zeros]

    def call(dev_args):
        outs = sharded(*dev_args)
        for o in outs:
            o.block_until_ready()
        return outs

    result = (make_args, call)
    _CACHE[key] = result
    return result


def _prep_in_maps(x, w_qkv, w_out):
    x = np.asarray(x, dtype=np.float32)
    w_qkv = np.asarray(w_qkv, dtype=np.float32)
    w_out = np.asarray(w_out, dtype=np.float32)
    in_maps = []
    xts = [np.ascontiguousarray(x[b].T) for b in range(B)]
    for core in range(8):
        b, g = divmod(core, 4)
        cl, ch = 256 * g, 256 * g + 256
        wqk = np.ascontiguousarray(
            np.concatenate([w_qkv[:, cl:ch], w_qkv[:, C + cl:C + ch]], axis=1))
        wv = np.ascontiguousarray(w_qkv[:, 2 * C + cl:2 * C + ch])
        wo = np.ascontiguousarray(w_out[cl:ch, :])
        in_maps.append({"xt": xts[b], "wqk": wqk, "wv": wv, "wo": wo,
                        "ones_c": np.ones((128, HL), dtype=np.float32)})
    return in_maps


def kernel(x, w_qkv, w_out):
    run = _get_runner()
    in_maps = _prep_in_maps(x, w_qkv, w_out)
    results = run(in_maps)
    y = np.zeros((B, T, C), dtype=np.float32)
    for core in range(8):
        b = core // 4
        y[b] += results[core]["y"]
    return y


if __name__ == "__main__":
    rng = np.random.default_rng(0)
    x = rng.standard_normal((B, T, C)).astype(np.float32)
    w_qkv = (rng.standard_normal((C, 3 * C)) / np.sqrt(C)).astype(np.float32)
    w_out = (rng.standard_normal((C, C)) / np.sqrt(C)).astype(np.float32)
    y = kernel(x=x, w_qkv=w_qkv, w_out=w_out)
    print("kernel ran, y:", y.shape, y.dtype, float(np.abs(y).max()))
